# revision 1
# baseline (speedup 1.0000x reference)
"""Trainium2 Bass kernel v3 for nn_Mixture_24541443129646.

loss = 0.5*S_xx - sum_n log sum_k exp(cross[n,k] - musq[k]/2)   (+ N*C shift)

Transposed layout (k on partitions), data-parallel over N on 8 cores:
  PE:   cross halves psum_h [128k, 1024n] = mupt_h^T @ xt chunk
  ACT:  e_h = Exp(psum_h + b_h), b_h = C - musq/2 per-partition bias
  DVE:  e01 = e0 + e1; drain rowsums psum -> SBUF
  PE:   rowsums rs [8, 1024] = ones^T @ e01
  DMA:  bounce rowsums through flat DRAM to respread n over partitions
  ACT:  Ln + accumulate
Host: shard/transpose/cast x, fold prec into mu, S_xx, final scalar.
"""

import sys

sys.path.insert(0, "/opt/trn_rl_repo")

from contextlib import ExitStack

import numpy as np

import concourse.bass as bass
import concourse.tile as tile
from concourse import mybir
from concourse.bass_utils import run_bass_kernel_spmd

N, K, D = 131072, 256, 128
NCORES = 8
RPC = N // NCORES
CW = 1024
NCHUNK = RPC // CW
MMW = 512
CSHIFT = 40.0

F32 = mybir.dt.float32
BF16 = mybir.dt.bfloat16
ACTF = mybir.ActivationFunctionType

XT_LOADS = [1024, 1024, 2048, 4096, 4096, 4096]
assert sum(XT_LOADS) == RPC


def _split_excess_waits(nc, max_waits=1):
    import bass_rust

    n_fix = 0
    for f in nc.m.functions:
        for bb in f.blocks:
            insts = bb.instructions
            out_list = []
            changed = False
            for ins in insts:
                si = ins.sync_info
                if si is not None and len(si.on_wait) > max_waits:
                    waits = list(si.on_wait)
                    extra, keep = waits[:-max_waits], waits[-max_waits:]
                    for i in range(0, len(extra), max_waits):
                        nd = mybir.InstDrain(name=f"I-waitfix-{n_fix}", ins=[], outs=[])
                        n_fix += 1
                        nd.engine = ins.engine
                        nd.sync_info = bass_rust.SyncInfo(
                            on_wait=extra[i : i + max_waits], on_update=[]
                        )
                        out_list.append(nd)
                    si.on_wait = keep
                    changed = True
                out_list.append(ins)
            if changed:
                bb.instructions = out_list
    return n_fix


def build_program(apply_waitfix=True):
    nc = bass.Bass("TRN2", target_bir_lowering=False, debug=False)

    xt = nc.dram_tensor("xt", [D, RPC], BF16, kind="ExternalInput").ap()
    mupt = nc.dram_tensor("mupt", [D, K], BF16, kind="ExternalInput").ap()
    bc = nc.dram_tensor("bc", [128, 2], F32, kind="ExternalInput").ap()
    o8 = nc.dram_tensor("o8", [128, 8], BF16, kind="ExternalInput").ap()
    rs_dram = nc.dram_tensor("rs_dram", [NCHUNK, 8, CW], F32, kind="Internal").ap()
    out = nc.dram_tensor("out", [128, 2], F32, kind="ExternalOutput").ap()

    with tile.TileContext(nc) as tc:
        with ExitStack() as ctx:
            cpool = ctx.enter_context(tc.tile_pool(name="const", bufs=1))
            xpool = ctx.enter_context(tc.tile_pool(name="xt", bufs=1))
            pA = ctx.enter_context(tc.tile_pool(name="pA", bufs=1, space="PSUM"))
            pB = ctx.enter_context(tc.tile_pool(name="pB", bufs=1, space="PSUM"))
            pR = ctx.enter_context(tc.tile_pool(name="pR", bufs=2, space="PSUM"))
            epool = ctx.enter_context(tc.tile_pool(name="e", bufs=4))
            e01pool = ctx.enter_context(tc.tile_pool(name="e01", bufs=3))
            rspool = ctx.enter_context(tc.tile_pool(name="rs", bufs=3))
            mpool = ctx.enter_context(tc.tile_pool(name="misc", bufs=1))

            # first xt chunk first: it gates the first exp
            xt_sb = []
            col = 0
            t0 = xpool.tile([D, XT_LOADS[0]], BF16, tag="xt0")
            nc.sync.dma_start(t0[:], xt[:, 0 : XT_LOADS[0]])
            xt_sb.append((t0, 0, XT_LOADS[0]))
            col = XT_LOADS[0]

            mupt_sb = cpool.tile([D, K], BF16, tag="mupt")
            nc.sync.dma_start(mupt_sb[:], mupt)
            bc_sb = cpool.tile([128, 2], F32, tag="bc")
            nc.sync.dma_start(bc_sb[:], bc)

            for li, w in enumerate(XT_LOADS[1:], start=1):
                t = xpool.tile([D, w], BF16, tag=f"xt{li}")
                nc.sync.dma_start(t[:], xt[:, col : col + w])
                xt_sb.append((t, col, w))
                col += w

            o8_sb = cpool.tile([128, 8], BF16, tag="o8")
            nc.sync.dma_start(o8_sb[:], o8)

            def xt_slice(c0, w):
                for t, s, n in xt_sb:
                    if s <= c0 and c0 + w <= s + n:
                        return t[:, c0 - s : c0 - s + w]
                raise AssertionError("chunk crosses load boundary")

            lnv = mpool.tile([128, NCHUNK * 8], F32, tag="lnv")
            out_sb = mpool.tile([128, 2], F32, tag="out")
            rsd = mpool.tile([128, NCHUNK * 8], F32, tag="rsd")

            # prewarm the Exp ACT table so the first exp skips the 1.3us load
            warm = mpool.tile([128, 1], F32, tag="warm")
            nc.vector.memset(warm[:], 0.0)
            nc.scalar.activation(warm[:], warm[:], ACTF.Exp)

            prev = None
            prev_reload = None
            for j in range(NCHUNK):
                psA = pA.tile([128, CW], F32, tag="A")
                psB = pB.tile([128, CW], F32, tag="B")
                for o in range(0, CW, MMW):
                    nc.tensor.matmul(
                        psA[:, o : o + MMW],
                        lhsT=mupt_sb[:, 0:128],
                        rhs=xt_slice(j * CW + o, MMW),
                        start=True,
                        stop=True,
                    )
                for o in range(0, CW, MMW):
                    nc.tensor.matmul(
                        psB[:, o : o + MMW],
                        lhsT=mupt_sb[:, 128:256],
                        rhs=xt_slice(j * CW + o, MMW),
                        start=True,
                        stop=True,
                    )
                e0 = epool.tile([128, CW], BF16, tag="e0")
                e1 = epool.tile([128, CW], BF16, tag="e1")
                nc.scalar.activation(e0[:], psA[:], ACTF.Exp, bias=bc_sb[:, 0:1])
                nc.scalar.activation(e1[:], psB[:], ACTF.Exp, bias=bc_sb[:, 1:2])
                e01 = e01pool.tile([128, CW], BF16, tag="e01")
                nc.vector.tensor_add(e01[:], e0[:], e1[:])

                if prev is not None:
                    _ones_drain_bounce(nc, o8_sb, rspool, pR, rs_dram, prev)
                if prev_reload is not None:
                    _reload(nc, rs_dram, rsd, prev_reload)
                prev_reload = prev[1] if prev is not None else None
                prev = (e01, j)

            _ones_drain_bounce(nc, o8_sb, rspool, pR, rs_dram, prev)
            if prev_reload is not None:
                _reload(nc, rs_dram, rsd, prev_reload)
            _reload(nc, rs_dram, rsd, prev[1])

            nc.scalar.activation(lnv[:], rsd[:], ACTF.Ln, accum_out=out_sb[:, 0:1])
            nc.vector.memset(out_sb[:, 1:2], 0.0)
            nc.sync.dma_start(out, out_sb[:])

    if apply_waitfix:
        _split_excess_waits(nc)
    return nc


def _ones_drain_bounce(nc, o8_sb, rspool, pR, rs_dram, prev):
    e01, j = prev
    rs = pR.tile([8, CW], F32, tag="rs")
    for o in range(0, CW, MMW):
        nc.tensor.matmul(
            rs[:, o : o + MMW],
            lhsT=o8_sb[:],
            rhs=e01[:, o : o + MMW],
            start=True,
            stop=True,
        )
    rssb = rspool.tile([8, CW], F32, tag="rssb")
    nc.vector.tensor_copy(rssb[:], rs[:])
    nc.sync.dma_start(rs_dram[j], rssb[:])


def _reload(nc, rs_dram, rsd, j):
    src = rs_dram[j, 0:1, :].rearrange("o (p i) -> (o p) i", p=128)
    nc.sync.dma_start(rsd[:, j * 8 : (j + 1) * 8], src)


def make_in_maps(x, mu, prec):
    import ml_dtypes

    x = np.asarray(x, dtype=np.float32)
    mu = np.asarray(mu, dtype=np.float32)
    prec = np.asarray(prec, dtype=np.float32)
    mupt = np.ascontiguousarray((mu * prec[None, :]).T).astype(ml_dtypes.bfloat16)
    musq_half = 0.5 * ((mu.astype(np.float64) ** 2) @ prec.astype(np.float64))
    bc = np.empty((128, 2), np.float32)
    bc[:, 0] = (CSHIFT - musq_half[0:128]).astype(np.float32)
    bc[:, 1] = (CSHIFT - musq_half[128:256]).astype(np.float32)
    o8 = np.ones((128, 8), np.float32).astype(ml_dtypes.bfloat16)
    in_maps = []
    for c in range(NCORES):
        xt_c = np.ascontiguousarray(x[c * RPC : (c + 1) * RPC, :].T).astype(
            ml_dtypes.bfloat16
        )
        in_maps.append({"xt": xt_c, "mupt": mupt, "bc": bc, "o8": o8})
    return in_maps


def combine_outputs(outs, x, prec):
    x64 = np.asarray(x, dtype=np.float64)
    prec64 = np.asarray(prec, dtype=np.float64)
    s_xx = float(((x64 * x64) @ prec64).sum())
    lse_sum = 0.0
    for o in outs:
        lse_sum += np.asarray(o, dtype=np.float64)[:, 0:2].sum()
    total = 0.5 * s_xx - (lse_sum - N * CSHIFT)
    return np.float32(total)


_CACHED_NC = None


def kernel(x, mu, prec):
    global _CACHED_NC
    if _CACHED_NC is None:
        _CACHED_NC = build_program()
    nc = _CACHED_NC
    in_maps = make_in_maps(x, mu, prec)
    res = run_bass_kernel_spmd(nc, in_maps, core_ids=list(range(NCORES)))
    outs = [res.results[c]["out"] for c in range(NCORES)]
    return combine_outputs(outs, x, prec)


if __name__ == "__main__":
    import reference

    inputs = {k: np.asarray(v) for k, v in reference.setup_inputs().items()}
    expected = float(reference.reference(**inputs))
    actual = float(kernel(**inputs))
    rel = abs(actual - expected) / max(1.0, abs(expected))
    print(f"expected={expected:.6f} actual={actual:.6f} rel={rel:.3e}")

